# revision 15
# baseline (speedup 1.0000x reference)
"""GroupedQueryAttention (head-axis-contracting variant) on 8 TRN2 NeuronCores.

Reference computation (B=2, S=2048, E=4096, D=128, H=32, Hkv=8, scale=4):
    q = einsum('bse,edh->bsdh', x, Wq) + bq          [B,S,D,H]
    k,v likewise with Hkv heads, then repeated 4x along h
    scores = einsum('bsdh,bseh->bsde', q, k) / sqrt(D)   (contracts the HEAD axis)
    out = softmax(scores, -1) @ v  -> reshape [B,S,E]

Because the head axis is contracted, q only enters through group-sums over the
4 q-heads sharing each kv head, and out's 4 head-columns per group are equal.
Per token the kernel computes:
    scoresT[e,d] = sum_g ksum... k[g,e] * qsum[g,d]        (K=8 matmul)
    E = exp(scoresT)                                        (no max-subtract;
                                                             |scores| < ~8)
    U[g|s, d] = [v | ones]^T @ E                            (K=128 matmul)
    out[t, d*32 + 4g+j] = U[g,d] / U[8,d]

Sharding: pure data-parallel over the 4096 tokens, 512 per core; weights
replicated. Host pre-work is layout/precision only (group-sum of Wq, bf16
casts, transposes); all FLOPs of the math above run on device.
"""

import os
import numpy as np
import ml_dtypes

_PHASES = os.environ.get("K_PHASES", "all")  # all | proj | nofin

import concourse.bass as bass
import concourse.mybir as mybir
import concourse.tile as tile
from concourse.vector_clock import ScopedClock

BF = ml_dtypes.bfloat16
F32 = mybir.dt.float32
BF16 = mybir.dt.bfloat16
AF = mybir.ActivationFunctionType

E, D, H, G, SC = 4096, 128, 32, 8, 4
B, S = 2, 2048
T = B * S
NCORES = 8
TPC = T // NCORES          # 512 tokens per core
KT = E // 128              # 32 contraction tiles
RCH = 32                   # stage-C / output token chunk
NCH = TPC // RCH           # 16 chunks


_MAXW = 1  # max sync-waits left on any one instruction


class _SplitDrainTileContext(tile.TileContext):
    """Workaround: this walrus build caps sync-wait commands per instruction.
    Spill excess waits onto same-engine nops inserted just before the
    instruction (same-engine stream order makes that equivalent), and do the
    same for the kernel-tail Drain."""

    def _add_instruction(self, inst):
        si = inst.sync_info
        if si is not None and si.on_wait and len(si.on_wait) > _MAXW:
            waits = list(si.on_wait)
            si.on_wait = waits[:_MAXW]
            for i in range(_MAXW, len(waits), _MAXW):
                nop = mybir.InstNoOp(
                    name=self.nc.get_next_instruction_name(),
                    engine=inst.engine, ins=[], outs=[],
                )
                nop.sync_info = mybir.SyncInfo(
                    on_wait=waits[i : i + _MAXW], on_update=[]
                )
                super()._add_instruction(nop)
        super()._add_instruction(inst)

    def _drain_and_barrier(self, tick_clock, wait_clock):
        nc = self.nc
        carrier = nc.sync.nop(nofuse=True).ins
        wait_clock.add_sem_waits(carrier, ScopedClock({None: tick_clock.global_clock}))
        waits = list(carrier.sync_info.on_wait) if carrier.sync_info else []
        if len(waits) > 1:
            carrier.sync_info.on_wait = waits[:1]
            for w in waits[1:]:
                extra = nc.sync.nop(nofuse=True).ins
                extra.sync_info = mybir.SyncInfo(on_wait=[w], on_update=[])
        nc.sync.drain()
        nc.all_engine_barrier()
        assert self.sems is not None
        popped = nc._tile_sem_poison_stack.pop()
        assert popped is self._sem_poison
        nc.clear_and_free_semaphores(list(self.sems.allocated().values()))
        nc.all_engine_barrier()


def _emit_body(nc, params, rep):
    """Emit one full forward pass. `params` maps name -> DRAM handle."""
    xw, wq, wk, wv, bq2, bk2, bv2, out_ext = (
        params["xw"], params["wq"], params["wk"], params["wv"],
        params["bq2"], params["bk2"], params["bv2"], params["out"],
    )
    tc = params["_tc"]
    with (
        tc.tile_pool(name=f"sbA{rep}", bufs=1) as sbA,
        tc.tile_pool(name=f"wp{rep}", bufs=2) as wpool,
        tc.tile_pool(name=f"pp{rep}", bufs=2, space="PSUM") as ppool,
        tc.tile_pool(name=f"gp{rep}", bufs=2) as gpool,
        tc.tile_pool(name=f"sp{rep}", bufs=2, space="PSUM") as spool,
        tc.tile_pool(name=f"up{rep}", bufs=2, space="PSUM") as upool,
        tc.tile_pool(name=f"ep{rep}", bufs=3) as epool,
        tc.tile_pool(name=f"ub{rep}", bufs=2) as ubpool,
        tc.tile_pool(name=f"fin{rep}", bufs=2) as fpool,
        tc.tile_pool(name=f"dr{rep}", bufs=1, space="DRAM") as dpool,
    ):
        # ---- resident inputs
        xsb = sbA.tile([128, KT * TPC], BF16, tag="xsb")       # [e_lo, (k, t)]
        nc.sync.dma_start(out=xsb[:], in_=xw[:])
        qsb = sbA.tile([128, G * TPC], BF16, tag="qsb")        # [d, (g, t)]
        ksb = sbA.tile([128, G * TPC], BF16, tag="ksb")
        vaug = sbA.tile([128, (G + 1) * TPC], BF16, tag="vaug")  # [dv,(g,t)]+ones
        nc.vector.memset(vaug[:, G * TPC :], 1.0)
        bq_sb = sbA.tile([128, G], F32, tag="bq_sb")
        bk_sb = sbA.tile([128, G], F32, tag="bk_sb")
        bv_sb = sbA.tile([128, G], F32, tag="bv_sb")
        nc.sync.dma_start(out=bq_sb[:], in_=bq2[:])
        nc.sync.dma_start(out=bk_sb[:], in_=bk2[:])
        nc.sync.dma_start(out=bv_sb[:], in_=bv2[:])

        # ---- projections: dest[:, g*TPC:(g+1)*TPC] = W_g^T @ xT (+ bias)
        for wext, dest, bias in ((wq, qsb, bq_sb), (wk, ksb, bk_sb), (wv, vaug, bv_sb)):
            for g in range(G):
                wtile = wpool.tile([128, KT * 128], BF16, tag="wtile")
                nc.sync.dma_start(out=wtile[:], in_=wext[g])
                psum = ppool.tile([128, TPC], F32, tag="psum")
                for k in range(KT):
                    nc.tensor.matmul(
                        psum[:],
                        wtile[:, k * 128 : (k + 1) * 128],
                        xsb[:, k * TPC : (k + 1) * TPC],
                        start=(k == 0),
                        stop=(k == KT - 1),
                    )
                nc.scalar.activation(
                    dest[:, g * TPC : (g + 1) * TPC], psum[:], AF.Identity,
                    bias=bias[:, g : g + 1],
                )

        # ---- bounce q/k through DRAM so stage-C gathers are 1 DMA each
        # (d-major layout: store order (d, g, t) matches qsb's linear order)
        q_dr = dpool.tile([D, G, TPC], BF16, tag="q_dr")
        k_dr = dpool.tile([D, G, TPC], BF16, tag="k_dr")
        a_dr = dpool.tile([NCH, D, RCH, G], F32, tag="a_dr")
        nc.sync.dma_start(out=q_dr[:], in_=qsb[:])
        nc.sync.dma_start(out=k_dr[:], in_=ksb[:])

        # ---- stage C, chunked over tokens
        for c in range(NCH if _PHASES != "proj" else 0):
            t0 = c * RCH
            # gather qg/kg [8 g, (d, t)] from DRAM (permuted DRAM-side AP)
            qg = gpool.tile([G, D * RCH], BF16, tag="qg")
            kg = gpool.tile([G, D * RCH], BF16, tag="kg")
            nc.sync.dma_start(
                out=qg[:], in_=q_dr[:, :, t0 : t0 + RCH].transpose([1, 0, 2])
            )
            nc.sync.dma_start(
                out=kg[:], in_=k_dr[:, :, t0 : t0 + RCH].transpose([1, 0, 2])
            )
            qgv = qg[:].rearrange("g (d t) -> g t d", t=RCH)
            kgv = kg[:].rearrange("g (d t) -> g t d", t=RCH)
            vv = vaug[:].rearrange("p (n t) -> p t n", t=TPC)
            # U' [128 d, 16-per-token (8 v-cols | s | pad)] packed chunk-wide
            ups2 = upool.tile([128, RCH * 16], F32, tag="ups2")
            for quad in range(RCH // 4):
                ps4 = spool.tile([128, 512], F32, tag="ps4")
                for i in range(4):
                    tl = quad * 4 + i
                    nc.tensor.matmul(
                        ps4[:, i * D : (i + 1) * D],
                        kgv[:, tl, :], qgv[:, tl, :],
                        start=True, stop=True,
                    )
                e4 = epool.tile([128, 512], BF16, tag="e4")
                nc.scalar.activation(e4[:], ps4[:], AF.Exp)
                for i in range(4):
                    tl = quad * 4 + i
                    nc.tensor.matmul(
                        ups2[:, tl * 16 : tl * 16 + 9],
                        e4[:, i * D : (i + 1) * D], vv[:, t0 + tl, :],
                        start=True, stop=True,
                    )

            # ---- finalize: one evacuation, normalize in d-major, transpose
            # via DRAM, duplicate 4x on the way out
            if _PHASES == "nofin":
                continue
            usb2 = ubpool.tile([128, RCH * 9], F32, tag="usb2")
            nc.vector.tensor_copy(
                usb2[:].rearrange("d (t s) -> d t s", s=9),
                ups2[:].rearrange("d (t s) -> d t s", s=16)[:, :, 0:9],
            )
            rtd = fpool.tile([128, RCH], F32, tag="rtd")
            uview = usb2[:].rearrange("d (t s) -> d t s", s=9)
            nc.vector.reciprocal(rtd[:], uview[:, :, 8])
            attn_n = fpool.tile([128, RCH * G], F32, tag="attn_n")
            nc.vector.tensor_tensor(
                attn_n[:].rearrange("d (t g) -> d t g", g=G),
                uview[:, :, 0:G],
                rtd[:].unsqueeze(2).broadcast_to([128, RCH, G]),
                op=mybir.AluOpType.mult,
            )
            nc.sync.dma_start(out=a_dr[c], in_=attn_n[:])
            atok = fpool.tile([RCH, D * G], F32, tag="atok")   # [t, (d, g)]
            nc.sync.dma_start(out=atok[:], in_=a_dr[c].transpose([1, 0, 2]))
            om = fpool.tile([RCH, D * H], F32, tag="om")
            nc.vector.tensor_copy(
                om[:].rearrange("t (d g j) -> t d g j", g=G, j=SC),
                atok[:].rearrange("t (d g) -> t d g", g=G)
                .unsqueeze(3).broadcast_to([RCH, D, G, SC]),
            )
            nc.sync.dma_start(out=out_ext[t0 : t0 + RCH, :], in_=om[:])


def build_program(reps=1):
    """Build the SPMD single-core program; same NEFF runs on all 8 cores."""
    nc = bass.Bass("TRN2", target_bir_lowering=False, debug=False,
                   num_devices=NCORES)
    params = {
        "xw": nc.declare_dram_parameter("xw", [128, KT, TPC], BF16, isOutput=False),
        "wq": nc.declare_dram_parameter("wq", [G, 128, KT, 128], BF16, isOutput=False),
        "wk": nc.declare_dram_parameter("wk", [G, 128, KT, 128], BF16, isOutput=False),
        "wv": nc.declare_dram_parameter("wv", [G, 128, KT, 128], BF16, isOutput=False),
        "bq2": nc.declare_dram_parameter("bq2", [128, G], F32, isOutput=False),
        "bk2": nc.declare_dram_parameter("bk2", [128, G], F32, isOutput=False),
        "bv2": nc.declare_dram_parameter("bv2", [128, G], F32, isOutput=False),
        "out": nc.declare_dram_parameter("out", [TPC, D * H], F32, isOutput=True),
    }
    with _SplitDrainTileContext(nc) as tc:
        params["_tc"] = tc
        for rep in range(reps):
            _emit_body(nc, params, rep)
    del params["_tc"]
    return nc


def prepare_inputs(x, Wq, bq, Wk, bk, Wv, bv):
    """Host-side sharding + layout/precision transforms -> per-core in_maps."""
    x = np.asarray(x, np.float32)
    scale = np.float32(1.0 / np.sqrt(D))

    def wmat(W, do_sum):
        W = np.asarray(W, np.float32)
        if do_sum:
            W = W.reshape(E, D, G, SC).sum(axis=3) * scale
        # [E, D, G] -> [E, g*128+d] -> [g, p, k, c] device tile layout
        m = W.transpose(0, 2, 1).reshape(E, G * D)
        return np.ascontiguousarray(
            m.reshape(KT, 128, G, D).transpose(2, 1, 0, 3)
        ).astype(BF)

    wq_h = wmat(Wq, True)
    wk_h = wmat(Wk, False)
    wv_h = wmat(Wv, False)
    bq_h = (np.asarray(bq, np.float32).reshape(D, G, SC).sum(axis=2) * scale)
    bk_h = np.ascontiguousarray(np.asarray(bk, np.float32))
    bv_h = np.ascontiguousarray(np.asarray(bv, np.float32))

    x_flat = x.reshape(T, E)
    in_maps = []
    for i in range(NCORES):
        xT = x_flat[i * TPC : (i + 1) * TPC].T          # [E, TPC]
        xw = xT.reshape(KT, 128, TPC).transpose(1, 0, 2).astype(BF)
        in_maps.append({
            "xw": np.ascontiguousarray(xw),
            "wq": wq_h, "wk": wk_h, "wv": wv_h,
            "bq2": bq_h, "bk2": bk_h, "bv2": bv_h,
        })
    return in_maps


def prepare_inputs_single(x, Wq, bq, Wk, bk, Wv, bv):
    """One-core variant for simulation: x must hold exactly TPC tokens."""
    x = np.asarray(x, np.float32).reshape(TPC, E)
    maps = prepare_inputs(
        np.broadcast_to(x.reshape(1, TPC, E), (NCORES, TPC, E)).reshape(B, S, E),
        Wq, bq, Wk, bk, Wv, bv,
    )
    return maps[0]


_CACHED = {}


def kernel(x, Wq, bq, Wk, bk, Wv, bv):
    from concourse.bass_utils import run_bass_kernel_spmd

    if "nc" not in _CACHED:
        _CACHED["nc"] = build_program(reps=1)
    nc = _CACHED["nc"]
    in_maps = prepare_inputs(x, Wq, bq, Wk, bk, Wv, bv)
    res = run_bass_kernel_spmd(nc, in_maps, list(range(NCORES)), trace=False)
    out = np.concatenate([res.results[i]["out"] for i in range(NCORES)], axis=0)
    return out.reshape(B, S, E).astype(np.float32)
